# revision 1
# baseline (speedup 1.0000x reference)
"""Local (windowed) attention with shared KV head — TRN2 Bass kernel.

Problem: b=1, L=4096, d_model=1024, n_head=16, d_head=64, w=512.
  qp = (q@Wq)/8; k,v = kv@Wkv; per 512-chunk attention over {prev,self,next}
  chunks with zero-padded edges (softmax includes exp(0)=1 terms for pads);
  out = ctx @ Wo.

Sharding: sequence-parallel over the 8 chunks, one chunk per NeuronCore.
Each core recomputes the K/V projection for its 3-chunk halo (no
collectives). Edge cores receive zero-filled halo slices, which reproduces
the reference's zero-padding exactly (scores 0 -> exp 1 in the softmax).

Per-core dataflow (all matmuls in float32r = full-rate PE, ~1e-4 rel err):
  kvp^T = [Wv|Wk]^T @ kv^T            (24 MMs)   -> vT (rows 0:64), kT (64:128)
  k3T2  = kT duplicated to both partition halves (SBUF->SBUF DMA)
  v65   = PE-transpose(vT) with a ones column appended   ([y,64+1] tiles)
  qp^T  = (Wq/8)^T @ q^T              (64 MMs)   -> 8 tiles [128,512], head pair per tile
  scores: S^T[y,x] per head, row-packed pairs (2 heads share the PE array)
  P^T   = exp(S^T) on ScalarE, PSUM->SBUF, [128,1024] groups
  ctx^T+Z = [v|1]^T @ P^T fused       (M=65: rows 0:64 ctx, row 64 = softmax denom)
  norm  : zinv=1/Z; broadcast via K=1 matmul; ctxn = ctx * zinv_bcast
  out   = ctxn^T-tiles (lhsT) @ Wo    (64 MMs)   -> [512,1024] row-major -> DMA
"""

import numpy as np

B, L, DM, NH, DH, W = 1, 4096, 1024, 16, 64, 512
NCORES = 8
CH = L // NCORES        # 512 tokens per core
YW = 3 * W              # 1536 halo positions
P = 128
NF = DM // P            # 8 feature tiles
NY = YW // P            # 12 y tiles
NPAIR = NH // 2         # 8 head pairs
NGRP = NY // 2          # 6 score groups of 2 y-tiles

_CACHE = {}


def _build():
    import concourse.mybir as mybir
    import concourse.tile as tile
    from concourse import bacc
    from concourse.masks import make_identity
    from contextlib import ExitStack

    F32 = mybir.dt.float32
    F32R = mybir.dt.float32r
    EXP = mybir.ActivationFunctionType.Exp

    nc = bacc.Bacc("TRN2", target_bir_lowering=False, debug=False)
    QT = nc.dram_tensor("QT", [DM, CH], F32R, kind="ExternalInput")
    KVT = nc.dram_tensor("KVT", [DM, YW], F32R, kind="ExternalInput")
    WQ = nc.dram_tensor("WQ", [DM, DM], F32R, kind="ExternalInput")     # pre-scaled by 1/8
    WVK = nc.dram_tensor("WVK", [DM, P], F32R, kind="ExternalInput")    # [Wv | Wk]
    WO = nc.dram_tensor("WO", [DM, DM], F32R, kind="ExternalInput")
    OUT = nc.dram_tensor("OUT", [CH, DM], F32, kind="ExternalOutput")

    with tile.TileContext(nc) as tc, ExitStack() as ctx:
        perm = ctx.enter_context(tc.tile_pool(name="perm", bufs=1))

        identf = perm.tile([P, P], F32, tag="identf")
        make_identity(nc, identf[:])
        onesb = perm.tile([P, P], F32R, tag="onesb")
        nc.vector.memset(onesb[:].bitcast(F32), 1.0)

        # --- persistent SBUF tiles
        wvk = [perm.tile([P, P], F32R, tag=f"wvk{f}", name=f"wvk{f}") for f in range(NF)]
        wq = [perm.tile([P, DM], F32R, tag=f"wq{f}", name=f"wq{f}") for f in range(NF)]
        wo = [perm.tile([P, DM], F32R, tag=f"wo{f}", name=f"wo{f}") for f in range(NF)]
        k3T2 = perm.tile([P, YW], F32R, tag="k3T2")
        vTs = perm.tile([64, YW], F32, tag="vTs")
        v65 = [perm.tile([P, 65], F32R, tag=f"v65_{t}", name=f"v65_{t}") for t in range(NY)]
        qpT = [perm.tile([P, CH], F32R, tag=f"qpT{m}", name=f"qpT{m}") for m in range(NF)]
        ctxn = [perm.tile([P, CH], F32R, tag=f"ctxn{i}", name=f"ctxn{i}") for i in range(NPAIR)]

        for f in range(NF):
            nc.sync.dma_start(wvk[f][:], WVK.ap()[P * f:P * (f + 1), :])

        with tc.tile_pool(name="kvt", bufs=1) as kvtp, \
             tc.tile_pool(name="ph0ps", bufs=3, space="PSUM") as ph0, \
             tc.tile_pool(name="tpps", bufs=2, space="PSUM") as tpp:
            kvt = [kvtp.tile([P, YW], F32R, tag=f"kvt{f}", name=f"kvt{f}") for f in range(NF)]
            for f in range(NF):
                nc.sync.dma_start(kvt[f][:], KVT.ap()[P * f:P * (f + 1), :])
            # kv projection: [128,512] psum per n-tile; rows 0:64=vT, 64:128=kT
            for n in range(3):
                ps = ph0.tile([P, W], F32, tag="kvp")
                for f in range(NF):
                    nc.tensor.matmul(ps[:], wvk[f][:], kvt[f][:, W * n:W * (n + 1)],
                                     start=(f == 0), stop=(f == NF - 1))
                ns = slice(W * n, W * (n + 1))
                nc.vector.tensor_copy(vTs[:, ns], ps[0:64, :])
                nc.vector.tensor_copy(k3T2[64:128, ns], ps[64:128, :])
            # duplicate kT into the low partition half (partition remap DMA)
            nc.sync.dma_start(k3T2[0:64, :], k3T2[64:128, :])
            # v65 tiles: PE transpose of vT + ones column
            for t in range(NY):
                tp = tpp.tile([P, 64], F32, tag="tp")
                nc.tensor.transpose(tp[:], vTs[:, P * t:P * (t + 1)],
                                    identf[0:64, 0:64])
                nc.vector.tensor_copy(v65[t][:, 0:64], tp[:])
                nc.vector.memset(v65[t][:, 64:65].bitcast(F32), 1.0)

        # --- q projection
        with tc.tile_pool(name="qt", bufs=1) as qtp, \
             tc.tile_pool(name="qpps", bufs=8, space="PSUM") as qpp:
            qt = [qtp.tile([P, CH], F32R, tag=f"qt{f}", name=f"qt{f}") for f in range(NF)]
            for f in range(NF):
                nc.sync.dma_start(qt[f][:], QT.ap()[P * f:P * (f + 1), :])
            for f in range(NF):
                nc.sync.dma_start(wq[f][:], WQ.ap()[P * f:P * (f + 1), :])
            for m in range(NF):
                ps = qpp.tile([P, CH], F32, tag="qp")
                for f in range(NF):
                    nc.tensor.matmul(ps[:], wq[f][:, P * m:P * (m + 1)], qt[f][:],
                                     start=(f == 0), stop=(f == NF - 1))
                nc.vector.tensor_copy(qpT[m][:], ps[:])

        for f in range(NF):
            nc.sync.dma_start(wo[f][:], WO.ap()[P * f:P * (f + 1), :])

        # --- attention per head pair
        with tc.tile_pool(name="scps", bufs=2, space="PSUM") as scp, \
             tc.tile_pool(name="cxps", bufs=3, space="PSUM") as cxp, \
             tc.tile_pool(name="pt", bufs=4) as ptp, \
             tc.tile_pool(name="zn", bufs=4) as znp:
            for i in range(NPAIR):
                cxA = cxp.tile([P, W], F32, tag="cx")
                cxB = cxp.tile([P, W], F32, tag="cx")
                for g in range(NGRP):
                    scA = scp.tile([P, 2 * W], F32, tag="sc")
                    scB = scp.tile([P, 2 * W], F32, tag="sc")
                    for t in range(2):
                        y = 2 * g + t
                        ys = slice(P * y, P * (y + 1))
                        ts_ = slice(W * t, W * (t + 1))
                        nc.tensor.matmul(scA[:, ts_], k3T2[0:64, ys],
                                         qpT[i][0:64, :], start=True, stop=True,
                                         tile_position=(0, 0))
                        nc.tensor.matmul(scB[:, ts_], k3T2[64:128, ys],
                                         qpT[i][64:128, :], start=True, stop=True,
                                         tile_position=(64, 0))
                    pA = ptp.tile([P, 2 * W], F32R, tag="pt")
                    pB = ptp.tile([P, 2 * W], F32R, tag="pt")
                    nc.scalar.activation(pA[:], scA[:], EXP)
                    nc.scalar.activation(pB[:], scB[:], EXP)
                    for t in range(2):
                        y = 2 * g + t
                        ts_ = slice(W * t, W * (t + 1))
                        st = (g == 0 and t == 0)
                        sp = (g == NGRP - 1 and t == 1)
                        nc.tensor.matmul(cxA[0:65, :], v65[y][:], pA[:, ts_],
                                         start=st, stop=sp)
                        nc.tensor.matmul(cxB[0:65, :], v65[y][:], pB[:, ts_],
                                         start=st, stop=sp)
                # normalize: ctxn[i][0:64] = cxA/Z_A ; [64:128] = cxB/Z_B (via DMA)
                for h, cx in ((0, cxA), (1, cxB)):
                    zinv = znp.tile([65, W], F32R, tag="zinv")
                    with nc.allow_low_precision(reason="softmax denom feeds f32r matmul"):
                        nc.vector.reciprocal(zinv[64:65, :], cx[64:65, :])
                    zbc = cxp.tile([P, W], F32, tag="cx")
                    nc.tensor.matmul(zbc[0:64, :], onesb[64:65, 0:64],
                                     zinv[64:65, :], start=True, stop=True,
                                     tile_position=(64, 0))
                    cxs = znp.tile([64, W], F32, tag="cxs")
                    nc.vector.tensor_copy(cxs[:], cx[0:64, :])
                    if h == 0:
                        with nc.allow_low_precision(reason="ctx feeds f32r matmul"):
                            nc.vector.tensor_mul(ctxn[i][0:64, :], cxs[:],
                                                 zbc[0:64, :])
                    else:
                        cbt = znp.tile([64, W], F32R, tag="cbt")
                        with nc.allow_low_precision(reason="ctx feeds f32r matmul"):
                            nc.vector.tensor_mul(cbt[:], cxs[:], zbc[0:64, :])
                        nc.sync.dma_start(ctxn[i][64:128, :], cbt[:])

        # --- output projection: out[x,o] = sum_i ctxn[i][:,x].T @ wo[i][:,o]
        with tc.tile_pool(name="opps", bufs=8, space="PSUM") as opp, \
             tc.tile_pool(name="osb", bufs=4) as osb:
            for x in range(4):
                xs = slice(P * x, P * (x + 1))
                for o in range(2):
                    os_ = slice(W * o, W * (o + 1))
                    ps = opp.tile([P, W], F32, tag="op")
                    for i in range(NPAIR):
                        nc.tensor.matmul(ps[:], ctxn[i][:, xs], wo[i][:, os_],
                                         start=(i == 0), stop=(i == NPAIR - 1))
                    ot = osb.tile([P, W], F32, tag="os")
                    nc.scalar.copy(ot[:], ps[:])
                    nc.sync.dma_start(OUT.ap()[xs, os_], ot[:])

    nc.compile()
    return nc


def _get_nc():
    if "nc" not in _CACHE:
        _CACHE["nc"] = _build()
    return _CACHE["nc"]


def kernel(q, kv, Wq, Wkv, Wo, w=None, _trace=False):
    from concourse import bass_utils

    q = np.asarray(q, np.float32).reshape(L, DM)
    kv = np.asarray(kv, np.float32).reshape(L, DM)
    Wq = np.asarray(Wq, np.float32)
    Wkv = np.asarray(Wkv, np.float32)
    Wo = np.asarray(Wo, np.float32)

    qT = np.ascontiguousarray(q.T)                      # [DM, L]
    kvT = np.ascontiguousarray(kv.T)                    # [DM, L]
    WQs = np.ascontiguousarray(Wq / np.sqrt(DH))        # fold 1/sqrt(d_head)
    WVK = np.ascontiguousarray(
        np.concatenate([Wkv[:, DH:], Wkv[:, :DH]], axis=1))  # [Wv | Wk]

    in_maps = []
    for c in range(NCORES):
        kvt_c = np.zeros((DM, YW), np.float32)
        lo = (c - 1) * CH
        hi = (c + 2) * CH
        src_lo, src_hi = max(lo, 0), min(hi, L)
        dst_lo = src_lo - lo
        kvt_c[:, dst_lo:dst_lo + (src_hi - src_lo)] = kvT[:, src_lo:src_hi]
        in_maps.append({
            "QT": np.ascontiguousarray(qT[:, c * CH:(c + 1) * CH]),
            "KVT": kvt_c,
            "WQ": WQs,
            "WVK": WVK,
            "WO": Wo,
        })

    nc = _get_nc()
    res = bass_utils.run_bass_kernel_spmd(
        nc, in_maps, core_ids=list(range(NCORES)), trace=_trace)
    if _trace:
        _CACHE["last_result"] = res

    out = np.concatenate([r["OUT"] for r in res.results], axis=0)
    return out.reshape(B, L, DM).astype(np.float32)



# revision 8
# speedup vs baseline: 2.0284x; 2.0284x over previous
"""Local (windowed) attention with shared KV head — TRN2 Bass kernel, v2.

Problem: b=1, L=4096, d_model=1024, n_head=16, d_head=64, w=512.
  qp = (q@Wq)/8; k,v = kv@Wkv; per 512-chunk attention over {prev,self,next}
  chunks with zero-padded edges (softmax includes exp(0)=1 terms for pads);
  out = ctx @ Wo.

Sharding: sequence-parallel over the 8 chunks, one chunk per NeuronCore.
Each core recomputes the K/V projection for its 3-chunk halo (no collectives).

v2 design (vs v1 baseline at ~381 us):
  - ScalarE exp is the roofline (~12.6M elements/core ~ 82 us at 1 elem/lane/cyc).
    The loop is restructured so exp streams continuously: per (head-pair, y-block)
    one [128,1024] PSUM score tile (A|B halves via row-tiled concurrent K=64
    matmuls) -> ONE wide ACTIVATE -> bf16 P tile -> two ctx matmuls (v65 trick:
    65th row of lhsT = ones accumulates the softmax denominator Z for free).
  - Scores double-buffered (2x2 PSUM banks) so ACT never waits on the PE.
  - bf16 everywhere on the main path: halves DMA bytes and SBUF footprint
    (PE rate is unchanged; accuracy ~1e-2 >> tolerance headroom).
  - reciprocal() on [1,512] cost 4 us each (64 us total) in v1; replaced by
    reciprocal_approx_fast on two rows of a packed tile (~5x faster per op).
  - q-projection for pair i+2 is interleaved into pair i's groups; normalization
    matmuls for pair i-1 are deferred into pair i's early groups, so the PE
    instruction stream never stalls the ACT stream at pair boundaries.
  - out-projection is a dense matmul tail.

PSUM banks in steady state: scores 2x[128,1024] (4) + cxA/cxB (2) + qp (1)
  + zbc (1) = 8.
"""

import numpy as np

B, L, DM, NH, DH, W = 1, 4096, 1024, 16, 64, 512
NCORES = 8
CH = L // NCORES        # 512 tokens per core
YW = 3 * W              # 1536 halo positions
P = 128
NF = DM // P            # 8 feature tiles
NY = YW // P            # 12 y blocks
NPAIR = NH // 2         # 8 head pairs

_CACHE = {}


def _build():
    import concourse.mybir as mybir
    import concourse.tile as tile
    from concourse import bacc
    from concourse.masks import make_identity
    from contextlib import ExitStack

    F32 = mybir.dt.float32
    F32R = mybir.dt.float32r
    BF16 = mybir.dt.bfloat16
    EXP = mybir.ActivationFunctionType.Exp

    nc = bacc.Bacc("TRN2", target_bir_lowering=False, debug=False)
    QT = nc.dram_tensor("QT", [DM, CH], BF16, kind="ExternalInput")
    KVT = nc.dram_tensor("KVT", [DM, YW], BF16, kind="ExternalInput")
    WQ = nc.dram_tensor("WQ", [DM, DM], BF16, kind="ExternalInput")     # pre-scaled by 1/8
    WVK = nc.dram_tensor("WVK", [DM, P], BF16, kind="ExternalInput")    # [Wv | Wk]
    WO = nc.dram_tensor("WO", [DM, DM], BF16, kind="ExternalInput")
    OUT = nc.dram_tensor("OUT", [CH, DM], F32, kind="ExternalOutput")

    with tile.TileContext(nc) as tc, ExitStack() as ctx, \
         nc.allow_low_precision(reason="bf16 datapath; rel-err budget 2e-2"):
        perm = ctx.enter_context(tc.tile_pool(name="perm", bufs=1))

        identb = perm.tile([64, 64], BF16, tag="identb")
        make_identity(nc, identb[:])
        # ones rows for the 1/Z broadcast matmuls (rows 0 and 32, 32-aligned
        # so tile_position can address them)
        onesEb = perm.tile([65, 64], BF16, tag="onesEb")
        nc.vector.memset(onesEb[64:65, :], 1.0)

        # --- persistent SBUF tiles (bf16)
        wvk = [perm.tile([P, P], BF16, tag=f"wvk{f}", name=f"wvk{f}") for f in range(NF)]
        wq = [perm.tile([P, DM], BF16, tag=f"wq{f}", name=f"wq{f}") for f in range(NF)]
        wo = [perm.tile([P, DM], BF16, tag=f"wo{f}", name=f"wo{f}") for f in range(NF)]
        qt = [perm.tile([P, CH], BF16, tag=f"qt{f}", name=f"qt{f}") for f in range(NF)]
        k3T2 = perm.tile([P, YW], BF16, tag="k3T2")
        vTs = perm.tile([64, YW], BF16, tag="vTs")
        v65 = [perm.tile([P, 65], BF16, tag=f"v65_{t}", name=f"v65_{t}") for t in range(NY)]
        qpT = [perm.tile([P, CH], BF16, tag=f"qpT{m}", name=f"qpT{m}") for m in range(NF)]
        ctxn = [perm.tile([P, CH], BF16, tag=f"ctxn{i}", name=f"ctxn{i}") for i in range(NPAIR)]

        for f in range(NF):
            nc.sync.dma_start(wvk[f][:], WVK.ap()[P * f:P * (f + 1), :])

        qpp = ctx.enter_context(tc.tile_pool(name="qpps", bufs=1, space="PSUM"))

        # ---------------- ramp: kv projection, v transposes, q projection
        with tc.tile_pool(name="kvt", bufs=1) as kvtp, \
             tc.tile_pool(name="ramp", bufs=2, space="PSUM") as rampp:
            kvt = [[kvtp.tile([P, W], BF16, tag=f"kvt{n}_{f}", name=f"kvt{n}_{f}") for f in range(NF)]
                   for n in range(3)]
            for n in range(3):
                for f in range(NF):
                    nc.sync.dma_start(kvt[n][f][:],
                                      KVT.ap()[P * f:P * (f + 1), W * n:W * (n + 1)])
            for f in range(NF):
                nc.sync.dma_start(qt[f][:], QT.ap()[P * f:P * (f + 1), :])
            for f in range(NF):
                nc.sync.dma_start(wq[f][:], WQ.ap()[P * f:P * (f + 1), :])

            # kv projection: [128,512] psum per chunk; rows 0:64=vT, 64:128=kT
            for n in range(3):
                ps = rampp.tile([P, W], F32, tag="kvp")
                for f in range(NF):
                    nc.tensor.matmul(ps[:], wvk[f][:], kvt[n][f][:],
                                     start=(f == 0), stop=(f == NF - 1))
                ns = slice(W * n, W * (n + 1))
                nc.vector.tensor_copy(vTs[:, ns], ps[0:64, :])
                nc.vector.tensor_copy(k3T2[64:128, ns], ps[64:128, :])
                # duplicate kT into the low partition half (partition remap DMA)
                nc.sync.dma_start(k3T2[0:64, ns], k3T2[64:128, ns])

            # v65 tiles: PE transpose of vT + ones column
            for t in range(NY):
                tp = rampp.tile([P, 64], BF16, tag="tp")
                nc.tensor.transpose(tp[:], vTs[:, P * t:P * (t + 1)], identb[:])
                nc.vector.tensor_copy(v65[t][:, 0:64], tp[:])
                nc.vector.memset(v65[t][:, 64:65], 1.0)

            # q projection for pairs 0 and 1 (rest interleaved into the loop)
            for m in range(2):
                ps = qpp.tile([P, CH], F32, tag="qp", name="qp")
                for f in range(NF):
                    nc.tensor.matmul(ps[:], wq[f][:, P * m:P * (m + 1)], qt[f][:],
                                     start=(f == 0), stop=(f == NF - 1))
                nc.vector.tensor_copy(qpT[m][:], ps[:])

        for f in range(NF):
            nc.sync.dma_start(wo[f][:], WO.ap()[P * f:P * (f + 1), :])

        # ---------------- attention main loop
        with tc.tile_pool(name="scps", bufs=2, space="PSUM") as scp, \
             tc.tile_pool(name="cxps", bufs=1, space="PSUM") as cxp, \
             tc.tile_pool(name="zbps", bufs=1, space="PSUM") as zbp, \
             tc.tile_pool(name="pt", bufs=3) as ptp, \
             tc.tile_pool(name="nrm", bufs=2) as nrm:

            pending_norm = [None]   # deferred zbc+mul emission for pair i-1

            for i in range(NPAIR):
                cxA = cxp.tile([65, W], F32, tag="cxA")
                cxB = cxp.tile([65, W], F32, tag="cxB")
                pg = [None] * NY
                qp_ps = [None]

                def emit_ctx(g, cxA=cxA, cxB=cxB, pg=pg):
                    st, sp = (g == 0), (g == NY - 1)
                    nc.tensor.matmul(cxA[:, :], v65[g][:], pg[g][:, 0:W],
                                     start=st, stop=sp)
                    nc.tensor.matmul(cxB[:, :], v65[g][:], pg[g][:, W:2 * W],
                                     start=st, stop=sp)

                for g in range(NY):
                    ys = slice(P * g, P * (g + 1))
                    scS = scp.tile([P, 2 * W], F32, tag="sc")
                    nc.tensor.matmul(scS[:, 0:W], k3T2[0:64, ys],
                                     qpT[i][0:64, :], start=True, stop=True,
                                     tile_position=(0, 0))
                    nc.tensor.matmul(scS[:, W:2 * W], k3T2[64:128, ys],
                                     qpT[i][64:128, :], start=True, stop=True,
                                     tile_position=(64, 0))
                    pt_ = ptp.tile([P, 2 * W], BF16, tag="pt")
                    nc.scalar.activation(pt_[:], scS[:], EXP)
                    pg[g] = pt_

                    if g >= 1:
                        emit_ctx(g - 1)
                    if g == 2 and pending_norm[0] is not None:
                        pending_norm[0]()
                        pending_norm[0] = None
                    # q projection for pair i+2, one f-tile per group
                    m = i + 2
                    if m < NPAIR and 3 <= g <= 10:
                        f = g - 3
                        if f == 0:
                            qp_ps[0] = qpp.tile([P, CH], F32, tag="qp", name="qp2")
                        nc.tensor.matmul(qp_ps[0][:], wq[f][:, P * m:P * (m + 1)],
                                         qt[f][:], start=(f == 0), stop=(f == NF - 1))
                    if m < NPAIR and g == 11:
                        nc.vector.tensor_copy(qpT[m][:], qp_ps[0][:])
                emit_ctx(NY - 1)

                # normalization prologue: evacuate Z + ctx from PSUM ASAP
                Zp = nrm.tile([65, 2 * W], F32, tag="Zp")
                zinv = nrm.tile([65, 2 * W], F32, tag="zinv")
                zinvb = nrm.tile([65, 2 * W], BF16, tag="zinvb")
                nc.vector.tensor_copy(Zp[64:65, 0:W], cxA[64:65, :])
                nc.vector.tensor_copy(Zp[64:65, W:2 * W], cxB[64:65, :])
                cxsA = nrm.tile([64, W], BF16, tag="cxsA")
                cxsB = nrm.tile([64, W], BF16, tag="cxsB")
                nc.vector.tensor_copy(cxsA[:], cxA[0:64, :])
                nc.vector.tensor_copy(cxsB[:], cxB[0:64, :])
                # custom DVE op requires base partition 0: run over all 65 rows
                # (rows 0:63 are don't-care lanes; row 64 holds Z_A|Z_B)
                nc.vector.reciprocal_approx_fast(zinv[:], Zp[:])
                nc.vector.tensor_copy(zinvb[64:65, :], zinv[64:65, :])

                def norm_tail(i=i, zinvb=zinvb, cxsA=cxsA, cxsB=cxsB):
                    zbA = zbp.tile([64, W], F32, tag="zb", name="zbA")
                    nc.tensor.matmul(zbA[:], onesEb[64:65, :], zinvb[64:65, 0:W],
                                     start=True, stop=True, tile_position=(64, 0))
                    nc.vector.tensor_mul(ctxn[i][0:64, :], cxsA[:], zbA[:])
                    zbB = zbp.tile([64, W], F32, tag="zb", name="zbB")
                    nc.tensor.matmul(zbB[:], onesEb[64:65, :], zinvb[64:65, W:2 * W],
                                     start=True, stop=True, tile_position=(64, 0))
                    cbt = nrm.tile([64, W], BF16, tag="cbt", name="cbt")
                    nc.vector.tensor_mul(cbt[:], cxsB[:], zbB[:])
                    nc.sync.dma_start(ctxn[i][64:128, :], cbt[:])

                pending_norm[0] = norm_tail

            pending_norm[0]()   # pair 7

        # ---------------- output projection tail
        with tc.tile_pool(name="opps", bufs=4, space="PSUM") as opp, \
             tc.tile_pool(name="osb", bufs=4) as osb:
            for x in range(4):
                xs = slice(P * x, P * (x + 1))
                for o in range(2):
                    os_ = slice(W * o, W * (o + 1))
                    ps = opp.tile([P, W], F32, tag="op")
                    for j in range(NPAIR):
                        nc.tensor.matmul(ps[:], ctxn[j][:, xs], wo[j][:, os_],
                                         start=(j == 0), stop=(j == NPAIR - 1))
                    ot = osb.tile([P, W], F32, tag="os")
                    nc.scalar.copy(ot[:], ps[:])
                    nc.sync.dma_start(OUT.ap()[xs, os_], ot[:])

    nc.compile()
    return nc


def _get_nc():
    if "nc" not in _CACHE:
        _CACHE["nc"] = _build()
    return _CACHE["nc"]


def kernel(q, kv, Wq, Wkv, Wo, w=None, _trace=False):
    import ml_dtypes
    from concourse import bass_utils

    BF = ml_dtypes.bfloat16

    q = np.asarray(q, np.float32).reshape(L, DM)
    kv = np.asarray(kv, np.float32).reshape(L, DM)
    Wq = np.asarray(Wq, np.float32)
    Wkv = np.asarray(Wkv, np.float32)
    Wo = np.asarray(Wo, np.float32)

    qT = np.ascontiguousarray(q.T.astype(BF))                 # [DM, L]
    kvT = np.ascontiguousarray(kv.T.astype(BF))               # [DM, L]
    WQs = np.ascontiguousarray((Wq / np.sqrt(DH)).astype(BF))  # fold 1/sqrt(d_head)
    WVK = np.ascontiguousarray(
        np.concatenate([Wkv[:, DH:], Wkv[:, :DH]], axis=1).astype(BF))  # [Wv | Wk]
    WOb = np.ascontiguousarray(Wo.astype(BF))

    in_maps = []
    for c in range(NCORES):
        kvt_c = np.zeros((DM, YW), BF)
        lo = (c - 1) * CH
        hi = (c + 2) * CH
        src_lo, src_hi = max(lo, 0), min(hi, L)
        dst_lo = src_lo - lo
        kvt_c[:, dst_lo:dst_lo + (src_hi - src_lo)] = kvT[:, src_lo:src_hi]
        in_maps.append({
            "QT": np.ascontiguousarray(qT[:, c * CH:(c + 1) * CH]),
            "KVT": kvt_c,
            "WQ": WQs,
            "WVK": WVK,
            "WO": WOb,
        })

    nc = _get_nc()
    res = bass_utils.run_bass_kernel_spmd(
        nc, in_maps, core_ids=list(range(NCORES)), trace=_trace)
    if _trace:
        _CACHE["last_result"] = res

    out = np.concatenate([r["OUT"] for r in res.results], axis=0)
    return out.reshape(B, L, DM).astype(np.float32)
